# revision 1
# baseline (speedup 1.0000x reference)
"""Dilated KNN (k=9, dilation=2) over query[4, 8192, 64] on 8 NeuronCores.

Sharding: batch b and query-half h per core (core = 2*b + h). Each core
computes scores s[m, n] = 2*x_m.x_n - |x_n|^2 for its 4096 queries against
all 8192 supports of its batch (equivalent ranking to squared euclidean
distance, negated so top-k smallest distance == top-k largest score), then
selects the top-17 per row with exact fp32 compare and lowest-index
tie-breaking, and emits indices of ranks 0, 2, ..., 16.

Matmul uses an fp32r hi/lo split (products exact in 11-bit chunks, fp32
PSUM accumulation) so the PE runs at full rate with fp32-class accuracy:
  MM1: [2ah; 2al] . [bh; bh]          (K=128)
  MM2: [2ah; 1; 1] . [bl; -sqh; -sql] (K=66, drops 2*al.bl ~ 1e-6)

Per 128-query tile:
  PE    : 2 x 16 fp32r matmuls -> PSUM
  ACT   : evict PSUM -> SBUF score tile [128, 8192]
  DVE   : per-512-chunk max8 (candidate values) + max_index (local indices),
          3-round merge (max8 / max_index / match_replace) over the 128
          candidates, then one fused scalar_tensor_tensor one-hot gather per
          output rank to map winning candidate positions to global indices.
"""

import sys
import types

import numpy as np

B = 4
N = 8192
C = 64
K_OUT = 9
NQ = N // 2
N_CORES = 8
CHUNK = 512
N_CHUNKS = N // CHUNK
NEG_BIG = -1.0e38


def _install_ntff_shim():
    """bass_utils imports antenv.axon_hooks for trace=True; the agent image
    lacks it. Register the ctypes-based hook so NTFF profiling works."""
    if "antenv.axon_hooks" in sys.modules:
        return
    try:
        from trn_agent_boot.trn_boot import _ntff_profile_via_ctypes

        hook = _ntff_profile_via_ctypes("/opt/axon/libaxon_pjrt.so")
        m = types.ModuleType("antenv.axon_hooks")
        m.get_axon_ntff_profile_hook = lambda: hook
        sys.modules["antenv.axon_hooks"] = m
    except Exception:
        pass


def build_kernel(nc, n_queries=NQ):
    import concourse.mybir as mybir
    import concourse.tile as tile
    from concourse import masks

    F32 = mybir.dt.float32
    F32R = mybir.dt.float32r
    U32 = mybir.dt.uint32
    I32 = mybir.dt.int32

    m_tiles = n_queries // 128
    xq = nc.dram_tensor("xq", [n_queries, C], F32, kind="ExternalInput")
    xs = nc.dram_tensor("xs", [N, C], F32, kind="ExternalInput")
    out = nc.dram_tensor("idx", [n_queries, K_OUT], I32, kind="ExternalOutput")

    with tile.TileContext(nc) as tc:
        with (
            tc.tile_pool(name="const", bufs=1) as constp,
            tc.tile_pool(name="big", bufs=1) as bigp,
        ):
            identity = constp.tile([128, 128], F32)
            masks.make_identity(nc, identity[:, :])
            iota_c = constp.tile([128, 128], F32)
            nc.gpsimd.iota(
                iota_c[:, :],
                pattern=[[1, 128]],
                base=0,
                channel_multiplier=0,
                allow_small_or_imprecise_dtypes=True,
            )
            base_f = constp.tile([128, 128], F32)
            nc.gpsimd.iota(
                base_f[:, :],
                pattern=[[CHUNK, N_CHUNKS], [0, 8]],
                base=0,
                channel_multiplier=0,
                allow_small_or_imprecise_dtypes=True,
            )
            ones64 = constp.tile([64, 1], F32)
            nc.vector.memset(ones64[:, :], 1.0)
            ones2 = constp.tile([2, CHUNK], F32)
            nc.vector.memset(ones2[:, :], 1.0)

            rhs1 = bigp.tile([128, N], F32R)
            rhs2 = bigp.tile([66, N], F32R)
            lhsT1 = bigp.tile([128, n_queries], F32R)
            lhsT2 = bigp.tile([66, n_queries], F32R)
            outbuf = bigp.tile([128, m_tiles * K_OUT], I32)

            with (
                tc.tile_pool(name="stage", bufs=4) as stagep,
                tc.tile_pool(name="dtmp", bufs=4) as dtmp,
                tc.tile_pool(name="ptr", bufs=4, space="PSUM") as ptrp,
                tc.tile_pool(name="psq", bufs=2, space="PSUM") as psqp,
            ):
                # query side first so the main loop's first tiles unblock early
                for j in range(n_queries // 128):
                    jsl = slice(j * 128, (j + 1) * 128)
                    st = stagep.tile([128, C], F32)
                    nc.sync.dma_start(st[:, :], xq.ap()[jsl, :])
                    pt = ptrp.tile([C, 128], F32)
                    nc.tensor.transpose(pt[:, :], st[:, :], identity[:, :])
                    nc.scalar.mul(lhsT1[0:64, jsl], pt[:, :], 2.0)  # 2ah
                    al = dtmp.tile([64, 128], F32, tag="al")
                    nc.vector.tensor_scalar(
                        al[:, :],
                        lhsT1[0:64, jsl].bitcast(F32),
                        -0.5,
                        None,
                        mybir.AluOpType.mult,
                    )
                    nc.vector.tensor_add(al[:, :], al[:, :], pt[:, :])  # a - ah
                    nc.scalar.mul(lhsT1[64:128, jsl], al[:, :], 2.0)  # 2al
                nc.sync.dma_start(
                    lhsT2[0:64, :].bitcast(F32), lhsT1[0:64, :].bitcast(F32)
                )
                nc.sync.dma_start(
                    lhsT2[64:66, :].bitcast(F32).rearrange("p (r c) -> p r c", c=CHUNK),
                    ones2[:, :].unsqueeze(1).broadcast_to(
                        [2, n_queries // CHUNK, CHUNK]
                    ),
                )

                # support side, grouped per 512-chunk
                for cc in range(N_CHUNKS):
                    sl = slice(cc * CHUNK, (cc + 1) * CHUNK)
                    sqcol = dtmp.tile([128, CHUNK // 128], F32, tag="sqcol")
                    sqscr = dtmp.tile([128, C], F32, tag="sqscr")
                    for k in range(CHUNK // 128):
                        j = cc * (CHUNK // 128) + k
                        jsl = slice(j * 128, (j + 1) * 128)
                        st = stagep.tile([128, C], F32)
                        nc.sync.dma_start(st[:, :], xs.ap()[jsl, :])
                        # |x_n|^2 per support row while it's still [n, c]
                        # (tensor_tensor_reduce hangs TRN2 here; use mul+reduce)
                        nc.vector.tensor_mul(sqscr[:, :], st[:, :], st[:, :])
                        nc.vector.reduce_sum(
                            sqcol[:, k : k + 1],
                            sqscr[:, :],
                            axis=mybir.AxisListType.X,
                        )
                        pt = ptrp.tile([C, 128], F32)
                        nc.tensor.transpose(pt[:, :], st[:, :], identity[:, :])
                        nc.scalar.copy(rhs1[0:64, jsl], pt[:, :])  # bh
                        bl = dtmp.tile([64, 128], F32, tag="bl")
                        nc.vector.tensor_sub(
                            bl[:, :], pt[:, :], rhs1[0:64, jsl].bitcast(F32)
                        )
                        nc.scalar.copy(rhs2[0:64, jsl], bl[:, :])  # bl
                    ptq = psqp.tile([CHUNK // 128, 128], F32)
                    nc.tensor.transpose(ptq[:, :], sqcol[:, :], identity[:, :])
                    sq4 = dtmp.tile([CHUNK // 128, 128], F32, tag="sq4")
                    nc.scalar.copy(sq4[:, :], ptq[:, :])
                    sqr = dtmp.tile([1, CHUNK], F32, tag="sqr")
                    for k in range(CHUNK // 128):
                        nc.sync.dma_start(
                            sqr[0:1, k * 128 : (k + 1) * 128], sq4[k : k + 1, :]
                        )
                    nsqh = dtmp.tile([1, CHUNK], F32R, tag="nsqh")
                    nc.scalar.mul(nsqh[:, :], sqr[:, :], -1.0)  # -sqh
                    nc.sync.dma_start(rhs2[64:65, sl], nsqh[:, :])
                    sql = dtmp.tile([1, CHUNK], F32, tag="sql")
                    nc.vector.tensor_add(sql[:, :], sqr[:, :], nsqh[:, :].bitcast(F32))
                    nsql = dtmp.tile([1, CHUNK], F32R, tag="nsql")
                    nc.scalar.mul(nsql[:, :], sql[:, :], -1.0)  # -sql
                    nc.sync.dma_start(rhs2[65:66, sl], nsql[:, :])
                    nc.sync.dma_start(
                        rhs1[64:128, sl].bitcast(F32), rhs1[0:64, sl].bitcast(F32)
                    )

            with (
                tc.tile_pool(name="spool", bufs=2) as spool,
                tc.tile_pool(name="cpool", bufs=2) as cpool,
                tc.tile_pool(name="pmm", bufs=8, space="PSUM") as pmm,
            ):
                for t in range(m_tiles):
                    qsl = slice(t * 128, (t + 1) * 128)
                    s_tile = spool.tile([128, N], F32, tag="s")
                    for cc in range(N_CHUNKS):
                        sl = slice(cc * CHUNK, (cc + 1) * CHUNK)
                        pm = pmm.tile([128, CHUNK], F32, tag="pm")
                        nc.tensor.matmul(
                            pm[:, :], lhsT1[:, qsl], rhs1[:, sl], start=True, stop=False
                        )
                        nc.tensor.matmul(
                            pm[:, :], lhsT2[:, qsl], rhs2[:, sl], start=False, stop=True
                        )
                        nc.scalar.copy(s_tile[:, sl], pm[:, :])

                    cand = cpool.tile([128, 128], F32, tag="cand")
                    candi = cpool.tile([128, 128], U32, tag="candi")
                    for cc in range(N_CHUNKS):
                        sl = slice(cc * CHUNK, (cc + 1) * CHUNK)
                        nc.vector.max(cand[:, cc * 8 : (cc + 1) * 8], s_tile[:, sl])
                    for cc in range(N_CHUNKS):
                        sl = slice(cc * CHUNK, (cc + 1) * CHUNK)
                        nc.vector.max_index(
                            candi[:, cc * 8 : (cc + 1) * 8],
                            cand[:, cc * 8 : (cc + 1) * 8],
                            s_tile[:, sl],
                        )

                    gidx_f = cpool.tile([128, 128], F32, tag="gidx")
                    gidx2 = cpool.tile([128, 128], F32, tag="gidx2")
                    nc.vector.tensor_copy(gidx_f[:, :], candi[:, :])
                    nc.vector.tensor_add(gidx2[:, :], gidx_f[:, :], base_f[:, :])

                    v24 = cpool.tile([128, 24], F32, tag="v24")
                    p24 = cpool.tile([128, 24], U32, tag="p24")
                    for r in range(3):
                        rsl = slice(r * 8, (r + 1) * 8)
                        nc.vector.max(v24[:, rsl], cand[:, :])
                        nc.vector.max_index(p24[:, rsl], v24[:, rsl], cand[:, :])
                        if r < 2:
                            nc.vector.match_replace(
                                cand[:, :], v24[:, rsl], cand[:, :], NEG_BIG
                            )

                    pos_f = cpool.tile([128, K_OUT], F32, tag="posf")
                    nc.vector.tensor_copy(pos_f[:, :], p24[:, 0:17:2])

                    scratch = cpool.tile([128, 128], F32, tag="scr")
                    o9 = cpool.tile([128, K_OUT], F32, tag="o9")
                    for j in range(K_OUT):
                        nc.vector.scalar_tensor_tensor(
                            scratch[:, :],
                            iota_c[:, :],
                            pos_f[:, j : j + 1],
                            gidx2[:, :],
                            mybir.AluOpType.is_equal,
                            mybir.AluOpType.mult,
                            accum_out=o9[:, j : j + 1],
                        )
                    nc.vector.tensor_copy(
                        outbuf[:, t * K_OUT : (t + 1) * K_OUT], o9[:, :]
                    )

            nc.sync.dma_start(
                out.ap().rearrange("(t p) j -> p t j", p=128),
                outbuf[:, :].rearrange("p (t j) -> p t j", j=K_OUT),
            )
    return nc


_COMPILED = None


def _get_compiled():
    global _COMPILED
    if _COMPILED is None:
        _install_ntff_shim()
        import concourse.bacc as bacc

        nc = bacc.Bacc("TRN2", target_bir_lowering=False, debug=False)
        build_kernel(nc)
        nc.compile()
        _COMPILED = nc
    return _COMPILED


LAST_RESULTS = None


def kernel(query: np.ndarray, _trace=False, _tmpdir=None) -> np.ndarray:
    global LAST_RESULTS
    from concourse import bass_utils

    query = np.ascontiguousarray(query, dtype=np.float32)
    assert query.shape == (B, N, C), query.shape
    nc = _get_compiled()

    in_maps = []
    for core in range(N_CORES):
        b, h = divmod(core, 2)
        in_maps.append(
            {
                "xq": query[b, h * NQ : (h + 1) * NQ, :],
                "xs": query[b],
            }
        )
    res = bass_utils.run_bass_kernel_spmd(
        nc, in_maps, core_ids=list(range(N_CORES)), trace=_trace, tmpdir=_tmpdir
    )
    LAST_RESULTS = res
    out = np.empty((B, N, K_OUT), np.int32)
    for core in range(N_CORES):
        b, h = divmod(core, 2)
        out[b, h * NQ : (h + 1) * NQ, :] = res.results[core]["idx"]
    return out



# revision 2
# speedup vs baseline: 1.1343x; 1.1343x over previous
"""Dilated KNN (k=9, dilation=2) over query[4, 8192, 64] on 8 NeuronCores.

Sharding: batch b and query-half h per core (core = 2*b + h). Each core
computes scores s[m, n] = 2*x_m.x_n - |x_n|^2 for its 4096 queries against
all 8192 supports of its batch, then selects the top-17 per row and emits
indices of ranks 0, 2, ..., 16.

Selection pipeline (single-scan, index-packed):
  PE  : fp32r hi/lo split matmul -> PSUM scores (exact products)
  ACT : u = uint32(Relu(-alpha2*s + 1.5*2^31)) -- the [2^31, 2^32) binade
        has ulp 256, so the cast's low 8 bits are zero for free; smaller u
        means better score
  POOL: p = u + n8 (n8 = column mod 256) -- packs the chunk-local index
        into the zeroed low byte; ties break toward the lowest index
  DVE : one MAX8 per 256-chunk on p bitcast to f32 (negative-float order
        reverses, so max8 finds the smallest u = best scores), then a
        3-round merge over the 256 candidates and int extraction:
        idx = (pos>>3)*256 + (p & 255). No second scan, no gathers.
"""

import sys
import types

import numpy as np

B = 4
N = 8192
C = 64
K_OUT = 9
NQ = N // 2
N_CORES = 8
PCHUNK = 512              # psum matmul chunk (columns per bank)
SCHUNK = 256              # selection chunk (8-bit local index)
SPAN = 2048               # ACT/pool span (4 psum chunks, 8 sel chunks)
N_SPANS = N // SPAN       # 4
ALPHA2 = 16000.0 * 256.0  # score quantization: delta = 256/ALPHA2 = 6.25e-5
OFFB = 1.5 * 2.0 ** 31
NEG_BIG = -1.0e38


def _install_ntff_shim():
    """bass_utils imports antenv.axon_hooks for trace=True; the agent image
    lacks it. Register the ctypes-based hook so NTFF profiling works."""
    if "antenv.axon_hooks" in sys.modules:
        return
    try:
        from trn_agent_boot.trn_boot import _ntff_profile_via_ctypes

        hook = _ntff_profile_via_ctypes("/opt/axon/libaxon_pjrt.so")
        m = types.ModuleType("antenv.axon_hooks")
        m.get_axon_ntff_profile_hook = lambda: hook
        sys.modules["antenv.axon_hooks"] = m
    except Exception:
        pass


def build_kernel(nc, n_queries=NQ):
    import concourse.mybir as mybir
    import concourse.tile as tile
    from concourse import masks

    F32 = mybir.dt.float32
    F32R = mybir.dt.float32r
    U32 = mybir.dt.uint32
    I32 = mybir.dt.int32

    m_tiles = n_queries // 128
    xq = nc.dram_tensor("xq", [n_queries, C], F32, kind="ExternalInput")
    xs = nc.dram_tensor("xs", [N, C], F32, kind="ExternalInput")
    out = nc.dram_tensor("idx", [n_queries, K_OUT], I32, kind="ExternalOutput")

    with tile.TileContext(nc) as tc:
        with (
            tc.tile_pool(name="const", bufs=1) as constp,
            tc.tile_pool(name="big", bufs=1) as bigp,
        ):
            identity = constp.tile([128, 128], F32)
            masks.make_identity(nc, identity[:, :])
            ones2 = constp.tile([2, PCHUNK], F32)
            nc.vector.memset(ones2[:, :], 1.0)

            # n8 span constant: col mod 256, as uint32
            n8f = constp.tile([128, SPAN], F32)
            nc.gpsimd.iota(
                n8f[:, :],
                pattern=[[0, SPAN // SCHUNK], [1, SCHUNK]],
                base=0,
                channel_multiplier=0,
                allow_small_or_imprecise_dtypes=True,
            )
            n8u = constp.tile([128, SPAN], U32)
            nc.vector.tensor_copy(n8u[:, :], n8f[:, :])

            offb = constp.tile([128, 1], F32)
            nc.vector.memset(offb[:, :], OFFB)
            m255 = constp.tile([128, 1], I32)
            nc.vector.memset(m255[:, :], 255)
            sh3 = constp.tile([128, 1], I32)
            nc.vector.memset(sh3[:, :], 3)
            sh8 = constp.tile([128, 1], I32)
            nc.vector.memset(sh8[:, :], 8)

            rhs1 = bigp.tile([128, N], F32R)
            rhs2 = bigp.tile([66, N], F32R)
            lhsT1 = bigp.tile([128, n_queries], F32R)
            lhsT2 = bigp.tile([66, n_queries], F32R)
            outbuf = bigp.tile([128, m_tiles * K_OUT], I32)

            with (
                tc.tile_pool(name="stage", bufs=4) as stagep,
                tc.tile_pool(name="dtmp", bufs=4) as dtmp,
                tc.tile_pool(name="ptr", bufs=4, space="PSUM") as ptrp,
                tc.tile_pool(name="psq", bufs=2, space="PSUM") as psqp,
            ):
                # query side first so the main loop's first tiles unblock early
                for j in range(n_queries // 128):
                    jsl = slice(j * 128, (j + 1) * 128)
                    st = stagep.tile([128, C], F32)
                    nc.sync.dma_start(st[:, :], xq.ap()[jsl, :])
                    pt = ptrp.tile([C, 128], F32)
                    nc.tensor.transpose(pt[:, :], st[:, :], identity[:, :])
                    nc.scalar.mul(lhsT1[0:64, jsl], pt[:, :], 2.0)  # 2ah
                    al = dtmp.tile([64, 128], F32, tag="al")
                    nc.vector.tensor_scalar(
                        al[:, :],
                        lhsT1[0:64, jsl].bitcast(F32),
                        -0.5,
                        None,
                        mybir.AluOpType.mult,
                    )
                    nc.vector.tensor_add(al[:, :], al[:, :], pt[:, :])  # a - ah
                    nc.scalar.mul(lhsT1[64:128, jsl], al[:, :], 2.0)  # 2al
                nc.sync.dma_start(
                    lhsT2[0:64, :].bitcast(F32), lhsT1[0:64, :].bitcast(F32)
                )
                nc.sync.dma_start(
                    lhsT2[64:66, :].bitcast(F32).rearrange("p (r c) -> p r c", c=PCHUNK),
                    ones2[:, :].unsqueeze(1).broadcast_to(
                        [2, n_queries // PCHUNK, PCHUNK]
                    ),
                )

                # support side, grouped per 512-chunk
                for cc in range(N // PCHUNK):
                    sl = slice(cc * PCHUNK, (cc + 1) * PCHUNK)
                    sqcol = dtmp.tile([128, PCHUNK // 128], F32, tag="sqcol")
                    sqscr = dtmp.tile([128, C], F32, tag="sqscr")
                    for k in range(PCHUNK // 128):
                        j = cc * (PCHUNK // 128) + k
                        jsl = slice(j * 128, (j + 1) * 128)
                        st = stagep.tile([128, C], F32)
                        nc.sync.dma_start(st[:, :], xs.ap()[jsl, :])
                        # |x_n|^2 per support row while it's still [n, c]
                        # (tensor_tensor_reduce hangs TRN2 here; use mul+reduce)
                        nc.vector.tensor_mul(sqscr[:, :], st[:, :], st[:, :])
                        nc.vector.reduce_sum(
                            sqcol[:, k : k + 1],
                            sqscr[:, :],
                            axis=mybir.AxisListType.X,
                        )
                        pt = ptrp.tile([C, 128], F32)
                        nc.tensor.transpose(pt[:, :], st[:, :], identity[:, :])
                        nc.scalar.copy(rhs1[0:64, jsl], pt[:, :])  # bh
                        bl = dtmp.tile([64, 128], F32, tag="bl")
                        nc.vector.tensor_sub(
                            bl[:, :], pt[:, :], rhs1[0:64, jsl].bitcast(F32)
                        )
                        nc.scalar.copy(rhs2[0:64, jsl], bl[:, :])  # bl
                    ptq = psqp.tile([PCHUNK // 128, 128], F32)
                    nc.tensor.transpose(ptq[:, :], sqcol[:, :], identity[:, :])
                    sq4 = dtmp.tile([PCHUNK // 128, 128], F32, tag="sq4")
                    nc.scalar.copy(sq4[:, :], ptq[:, :])
                    sqr = dtmp.tile([1, PCHUNK], F32, tag="sqr")
                    for k in range(PCHUNK // 128):
                        nc.sync.dma_start(
                            sqr[0:1, k * 128 : (k + 1) * 128], sq4[k : k + 1, :]
                        )
                    nsqh = dtmp.tile([1, PCHUNK], F32R, tag="nsqh")
                    nc.scalar.mul(nsqh[:, :], sqr[:, :], -1.0)  # -sqh
                    nc.sync.dma_start(rhs2[64:65, sl], nsqh[:, :])
                    sql = dtmp.tile([1, PCHUNK], F32, tag="sql")
                    nc.vector.tensor_add(sql[:, :], sqr[:, :], nsqh[:, :].bitcast(F32))
                    nsql = dtmp.tile([1, PCHUNK], F32R, tag="nsql")
                    nc.scalar.mul(nsql[:, :], sql[:, :], -1.0)  # -sql
                    nc.sync.dma_start(rhs2[65:66, sl], nsql[:, :])
                    nc.sync.dma_start(
                        rhs1[64:128, sl].bitcast(F32), rhs1[0:64, sl].bitcast(F32)
                    )

            with (
                tc.tile_pool(name="upool", bufs=3) as upool,
                tc.tile_pool(name="ppool", bufs=3) as ppool,
                tc.tile_pool(name="cpool", bufs=2) as cpool,
                tc.tile_pool(name="pmm", bufs=8, space="PSUM") as pmm,
            ):
                for t in range(m_tiles):
                    qsl = slice(t * 128, (t + 1) * 128)
                    cand = cpool.tile([128, 256], F32, tag="cand")
                    for j in range(N_SPANS):
                        u = upool.tile([128, SPAN], U32, tag="u")
                        for k in range(SPAN // PCHUNK):
                            cc = j * (SPAN // PCHUNK) + k
                            sl = slice(cc * PCHUNK, (cc + 1) * PCHUNK)
                            pm = pmm.tile([128, PCHUNK], F32, tag="pm")
                            nc.tensor.matmul(
                                pm[:, :], lhsT1[:, qsl], rhs1[:, sl],
                                start=True, stop=False,
                            )
                            nc.tensor.matmul(
                                pm[:, :], lhsT2[:, qsl], rhs2[:, sl],
                                start=False, stop=True,
                            )
                            nc.scalar.activation(
                                u[:, k * PCHUNK : (k + 1) * PCHUNK],
                                pm[:, :],
                                mybir.ActivationFunctionType.Relu,
                                bias=offb[:, 0:1],
                                scale=-ALPHA2,
                            )
                        p = ppool.tile([128, SPAN], U32, tag="p")
                        nc.gpsimd.tensor_tensor(
                            p[:, :], u[:, :], n8u[:, :], mybir.AluOpType.add
                        )
                        for h in range(SPAN // SCHUNK):
                            g = j * (SPAN // SCHUNK) + h
                            nc.vector.max(
                                cand[:, g * 8 : (g + 1) * 8],
                                p[:, h * SCHUNK : (h + 1) * SCHUNK].bitcast(F32),
                            )

                    v24 = cpool.tile([128, 24], F32, tag="v24")
                    p24 = cpool.tile([128, 24], U32, tag="p24")
                    for r in range(3):
                        rsl = slice(r * 8, (r + 1) * 8)
                        nc.vector.max(v24[:, rsl], cand[:, :])
                        nc.vector.max_index(p24[:, rsl], v24[:, rsl], cand[:, :])
                        if r < 2:
                            nc.vector.match_replace(
                                cand[:, :], v24[:, rsl], cand[:, :], NEG_BIG
                            )

                    # idx = (pos>>3)*256 + (packed & 255)
                    n9 = cpool.tile([128, K_OUT], I32, tag="n9")
                    nc.vector.tensor_scalar(
                        n9[:, :],
                        v24[:, 0:17:2].bitcast(I32),
                        m255[:, 0:1],
                        None,
                        mybir.AluOpType.bitwise_and,
                    )
                    b9 = cpool.tile([128, K_OUT], I32, tag="b9")
                    nc.vector.tensor_scalar(
                        b9[:, :],
                        p24[:, 0:17:2].bitcast(I32),
                        sh3[:, 0:1],
                        sh8[:, 0:1],
                        mybir.AluOpType.logical_shift_right,
                        mybir.AluOpType.logical_shift_left,
                    )
                    nc.vector.tensor_add(
                        outbuf[:, t * K_OUT : (t + 1) * K_OUT], b9[:, :], n9[:, :]
                    )

            nc.sync.dma_start(
                out.ap().rearrange("(t p) j -> p t j", p=128),
                outbuf[:, :].rearrange("p (t j) -> p t j", j=K_OUT),
            )
    return nc


_COMPILED = None


def _get_compiled():
    global _COMPILED
    if _COMPILED is None:
        _install_ntff_shim()
        import concourse.bacc as bacc

        nc = bacc.Bacc("TRN2", target_bir_lowering=False, debug=False)
        build_kernel(nc)
        nc.compile()
        _COMPILED = nc
    return _COMPILED


LAST_RESULTS = None


def kernel(query: np.ndarray, _trace=False, _tmpdir=None) -> np.ndarray:
    global LAST_RESULTS
    from concourse import bass_utils

    query = np.ascontiguousarray(query, dtype=np.float32)
    assert query.shape == (B, N, C), query.shape
    nc = _get_compiled()

    in_maps = []
    for core in range(N_CORES):
        b, h = divmod(core, 2)
        in_maps.append(
            {
                "xq": query[b, h * NQ : (h + 1) * NQ, :],
                "xs": query[b],
            }
        )
    res = bass_utils.run_bass_kernel_spmd(
        nc, in_maps, core_ids=list(range(N_CORES)), trace=_trace, tmpdir=_tmpdir
    )
    LAST_RESULTS = res
    out = np.empty((B, N, K_OUT), np.int32)
    for core in range(N_CORES):
        b, h = divmod(core, 2)
        out[b, h * NQ : (h + 1) * NQ, :] = res.results[core]["idx"]
    return out


# revision 6
# speedup vs baseline: 1.2479x; 1.1001x over previous
"""Dilated KNN (k=9, dilation=2) over query[4, 8192, 64] on 8 NeuronCores.

Sharding: batch b and query-half h per core (core = 2*b + h). Each core
computes scores s[m, n] = 2*x_m.x_n - |x_n|^2 for its 4096 queries against
all 8192 supports of its batch, then selects the top-17 per row and emits
indices of ranks 0, 2, ..., 16.

Selection pipeline (single-scan, index-packed):
  PE  : fp32r hi/lo split matmul -> PSUM scores (exact products)
  ACT : u = uint32(Relu(-alpha2*s + 1.5*2^31)) -- the [2^31, 2^32) binade
        has ulp 256, so the cast's low 8 bits are zero for free; smaller u
        means better score
  POOL: p = u + n8 (n8 = column mod 256) -- packs the chunk-local index
        into the zeroed low byte; ties break toward the lowest index
  DVE : one MAX8 per 256-chunk on p bitcast to f32 (negative-float order
        reverses, so max8 finds the smallest u = best scores), then a
        3-round merge over the 256 candidates and int extraction:
        idx = (pos>>3)*256 + (p & 255). No second scan, no gathers.
"""

import sys
import types

import numpy as np

B = 4
N = 8192
C = 64
K_OUT = 9
NQ = N // 2
N_CORES = 8
PCHUNK = 512              # psum matmul chunk (columns per bank)
SCHUNK = 256              # selection chunk (8-bit local index)
SPAN = 2048               # ACT/pool span (4 psum chunks, 8 sel chunks)
N_SPANS = N // SPAN       # 4
ALPHA2 = 16000.0 * 256.0  # score quantization: delta = 256/ALPHA2 = 6.25e-5
OFFB = 1.5 * 2.0 ** 31
NEG_BIG = -1.0e38


def _install_ntff_shim():
    """bass_utils imports antenv.axon_hooks for trace=True; the agent image
    lacks it. Register the ctypes-based hook so NTFF profiling works."""
    if "antenv.axon_hooks" in sys.modules:
        return
    try:
        from trn_agent_boot.trn_boot import _ntff_profile_via_ctypes

        hook = _ntff_profile_via_ctypes("/opt/axon/libaxon_pjrt.so")
        m = types.ModuleType("antenv.axon_hooks")
        m.get_axon_ntff_profile_hook = lambda: hook
        sys.modules["antenv.axon_hooks"] = m
    except Exception:
        pass


def build_kernel(nc, n_queries=NQ):
    import concourse.mybir as mybir
    import concourse.tile as tile
    from concourse import masks

    F32 = mybir.dt.float32
    F32R = mybir.dt.float32r
    U32 = mybir.dt.uint32
    I32 = mybir.dt.int32

    m_tiles = n_queries // 128
    xq = nc.dram_tensor("xq", [n_queries, C], F32, kind="ExternalInput")
    xs = nc.dram_tensor("xs", [N, C], F32, kind="ExternalInput")
    # packed winner values and candidate positions; host decodes
    # idx = (pos >> 3) * 256 + (val & 255)
    outv = nc.dram_tensor("pkv", [n_queries, K_OUT], I32, kind="ExternalOutput")
    outp = nc.dram_tensor("pkp", [n_queries, K_OUT], I32, kind="ExternalOutput")

    with tile.TileContext(nc) as tc:
        with (
            tc.tile_pool(name="const", bufs=1) as constp,
            tc.tile_pool(name="big", bufs=1) as bigp,
        ):
            identity = constp.tile([128, 128], F32)
            masks.make_identity(nc, identity[:, :])
            ones2 = constp.tile([2, PCHUNK], F32)
            nc.vector.memset(ones2[:, :], 1.0)

            # n8 span constant: col mod 256, as uint32
            n8f = constp.tile([128, SPAN], F32)
            nc.gpsimd.iota(
                n8f[:, :],
                pattern=[[0, SPAN // SCHUNK], [1, SCHUNK]],
                base=0,
                channel_multiplier=0,
                allow_small_or_imprecise_dtypes=True,
            )
            n8u = constp.tile([128, SPAN], U32)
            nc.vector.tensor_copy(n8u[:, :], n8f[:, :])

            offb = constp.tile([128, 1], F32)
            nc.vector.memset(offb[:, :], OFFB)

            rhs1 = bigp.tile([128, N], F32R)
            rhs2 = bigp.tile([66, N], F32R)
            lhsT1 = bigp.tile([128, n_queries], F32R)
            lhsT2 = bigp.tile([66, n_queries], F32R)

            with (
                tc.tile_pool(name="stage", bufs=4) as stagep,
                tc.tile_pool(name="dtmp", bufs=4) as dtmp,
                tc.tile_pool(name="ptr", bufs=4, space="PSUM") as ptrp,
                tc.tile_pool(name="psq", bufs=2, space="PSUM") as psqp,
            ):
                # query side first so the main loop's first tiles unblock early
                for j in range(n_queries // 128):
                    jsl = slice(j * 128, (j + 1) * 128)
                    st = stagep.tile([128, C], F32)
                    nc.sync.dma_start(st[:, :], xq.ap()[jsl, :])
                    pt = ptrp.tile([C, 128], F32)
                    nc.tensor.transpose(pt[:, :], st[:, :], identity[:, :])
                    nc.scalar.mul(lhsT1[0:64, jsl], pt[:, :], 2.0)  # 2ah
                    al = dtmp.tile([64, 128], F32, tag="al")
                    nc.vector.tensor_scalar(
                        al[:, :],
                        lhsT1[0:64, jsl].bitcast(F32),
                        -0.5,
                        None,
                        mybir.AluOpType.mult,
                    )
                    nc.vector.tensor_add(al[:, :], al[:, :], pt[:, :])  # a - ah
                    nc.scalar.mul(lhsT1[64:128, jsl], al[:, :], 2.0)  # 2al
                nc.sync.dma_start(
                    lhsT2[0:64, :].bitcast(F32), lhsT1[0:64, :].bitcast(F32)
                )
                nc.sync.dma_start(
                    lhsT2[64:66, :].bitcast(F32).rearrange("p (r c) -> p r c", c=PCHUNK),
                    ones2[:, :].unsqueeze(1).broadcast_to(
                        [2, n_queries // PCHUNK, PCHUNK]
                    ),
                )

                # support side, grouped per 512-chunk
                for cc in range(N // PCHUNK):
                    sl = slice(cc * PCHUNK, (cc + 1) * PCHUNK)
                    sqcol = dtmp.tile([128, PCHUNK // 128], F32, tag="sqcol")
                    sqscr = dtmp.tile([128, C], F32, tag="sqscr")
                    for k in range(PCHUNK // 128):
                        j = cc * (PCHUNK // 128) + k
                        jsl = slice(j * 128, (j + 1) * 128)
                        st = stagep.tile([128, C], F32)
                        nc.sync.dma_start(st[:, :], xs.ap()[jsl, :])
                        # |x_n|^2 per support row while it's still [n, c]
                        # (tensor_tensor_reduce hangs TRN2 here; use mul+reduce)
                        nc.vector.tensor_mul(sqscr[:, :], st[:, :], st[:, :])
                        nc.vector.reduce_sum(
                            sqcol[:, k : k + 1],
                            sqscr[:, :],
                            axis=mybir.AxisListType.X,
                        )
                        pt = ptrp.tile([C, 128], F32)
                        nc.tensor.transpose(pt[:, :], st[:, :], identity[:, :])
                        nc.scalar.copy(rhs1[0:64, jsl], pt[:, :])  # bh
                        bl = dtmp.tile([64, 128], F32, tag="bl")
                        nc.vector.tensor_sub(
                            bl[:, :], pt[:, :], rhs1[0:64, jsl].bitcast(F32)
                        )
                        nc.scalar.copy(rhs2[0:64, jsl], bl[:, :])  # bl
                    ptq = psqp.tile([PCHUNK // 128, 128], F32)
                    nc.tensor.transpose(ptq[:, :], sqcol[:, :], identity[:, :])
                    sq4 = dtmp.tile([PCHUNK // 128, 128], F32, tag="sq4")
                    nc.scalar.copy(sq4[:, :], ptq[:, :])
                    sqr = dtmp.tile([1, PCHUNK], F32, tag="sqr")
                    for k in range(PCHUNK // 128):
                        nc.sync.dma_start(
                            sqr[0:1, k * 128 : (k + 1) * 128], sq4[k : k + 1, :]
                        )
                    nsqh = dtmp.tile([1, PCHUNK], F32R, tag="nsqh")
                    nc.scalar.mul(nsqh[:, :], sqr[:, :], -1.0)  # -sqh
                    nc.sync.dma_start(rhs2[64:65, sl], nsqh[:, :])
                    sql = dtmp.tile([1, PCHUNK], F32, tag="sql")
                    nc.vector.tensor_add(sql[:, :], sqr[:, :], nsqh[:, :].bitcast(F32))
                    nsql = dtmp.tile([1, PCHUNK], F32R, tag="nsql")
                    nc.scalar.mul(nsql[:, :], sql[:, :], -1.0)  # -sql
                    nc.sync.dma_start(rhs2[65:66, sl], nsql[:, :])
                    nc.sync.dma_start(
                        rhs1[64:128, sl].bitcast(F32), rhs1[0:64, sl].bitcast(F32)
                    )

            with (
                tc.tile_pool(name="upool", bufs=3) as upool,
                tc.tile_pool(name="ppool", bufs=3) as ppool,
                tc.tile_pool(name="cpool", bufs=2) as cpool,
                tc.tile_pool(name="pmm", bufs=8, space="PSUM") as pmm,
            ):
                for t in range(m_tiles):
                    qsl = slice(t * 128, (t + 1) * 128)
                    cand = cpool.tile([128, 256], F32, tag="cand")
                    for j in range(N_SPANS):
                        u = upool.tile([128, SPAN], U32, tag="u")
                        for k in range(SPAN // PCHUNK):
                            cc = j * (SPAN // PCHUNK) + k
                            sl = slice(cc * PCHUNK, (cc + 1) * PCHUNK)
                            pm = pmm.tile([128, PCHUNK], F32, tag="pm")
                            nc.tensor.matmul(
                                pm[:, :], lhsT1[:, qsl], rhs1[:, sl],
                                start=True, stop=False,
                            )
                            nc.tensor.matmul(
                                pm[:, :], lhsT2[:, qsl], rhs2[:, sl],
                                start=False, stop=True,
                            )
                            nc.scalar.activation(
                                u[:, k * PCHUNK : (k + 1) * PCHUNK],
                                pm[:, :],
                                mybir.ActivationFunctionType.Relu,
                                bias=offb[:, 0:1],
                                scale=-ALPHA2,
                            )
                        p = ppool.tile([128, SPAN], U32, tag="p")
                        nc.gpsimd.tensor_tensor(
                            p[:, :], u[:, :], n8u[:, :], mybir.AluOpType.add
                        )
                        for h in range(SPAN // SCHUNK):
                            g = j * (SPAN // SCHUNK) + h
                            nc.vector.max(
                                cand[:, g * 8 : (g + 1) * 8],
                                p[:, h * SCHUNK : (h + 1) * SCHUNK].bitcast(F32),
                            )

                    v24 = cpool.tile([128, 24], F32, tag="v24")
                    p24 = cpool.tile([128, 24], U32, tag="p24")
                    for r in range(3):
                        rsl = slice(r * 8, (r + 1) * 8)
                        nc.vector.max(v24[:, rsl], cand[:, :])
                        nc.vector.max_index(p24[:, rsl], v24[:, rsl], cand[:, :])
                        if r < 2:
                            nc.vector.match_replace(
                                cand[:, :], v24[:, rsl], cand[:, :], NEG_BIG
                            )

                    qrow = slice(t * 128, (t + 1) * 128)
                    nc.sync.dma_start(
                        outv.ap()[qrow, :], v24[:, 0:17:2].bitcast(I32)
                    )
                    nc.sync.dma_start(
                        outp.ap()[qrow, :], p24[:, 0:17:2].bitcast(I32)
                    )
    return nc


_COMPILED = None


def _get_compiled():
    global _COMPILED
    if _COMPILED is None:
        _install_ntff_shim()
        import concourse.bacc as bacc

        nc = bacc.Bacc("TRN2", target_bir_lowering=False, debug=False)
        build_kernel(nc)
        nc.compile()
        _COMPILED = nc
    return _COMPILED


LAST_RESULTS = None


def kernel(query: np.ndarray, _trace=False, _tmpdir=None) -> np.ndarray:
    global LAST_RESULTS
    from concourse import bass_utils

    query = np.ascontiguousarray(query, dtype=np.float32)
    assert query.shape == (B, N, C), query.shape
    nc = _get_compiled()

    in_maps = []
    for core in range(N_CORES):
        b, h = divmod(core, 2)
        in_maps.append(
            {
                "xq": query[b, h * NQ : (h + 1) * NQ, :],
                "xs": query[b],
            }
        )
    res = bass_utils.run_bass_kernel_spmd(
        nc, in_maps, core_ids=list(range(N_CORES)), trace=_trace, tmpdir=_tmpdir
    )
    LAST_RESULTS = res
    out = np.empty((B, N, K_OUT), np.int32)
    for core in range(N_CORES):
        b, h = divmod(core, 2)
        pv = res.results[core]["pkv"].view(np.uint32)
        pp = res.results[core]["pkp"].view(np.uint32)
        idx = (pp >> 3) * SCHUNK + (pv & (SCHUNK - 1))
        out[b, h * NQ : (h + 1) * NQ, :] = idx.astype(np.int32)
    return out
